# revision 19
# baseline (speedup 1.0000x reference)
"""Bidirectional spatial Mamba block on 8 TRN2 NeuronCores.

Sharding: data-parallel over batch (16 samples -> 2 per core), params
replicated. Each core runs the full block on its 2 local samples.

Device layout:
  - F-layout: [d partitions (4 tiles of 128), free = (b, t)]. Scan-phase
    F tensors are 2050 wide: cols [0,1024) sample 0, [1024,1026) zero
    gap, [1026,2050) sample 1. The gap carries a=0, u=0 so one
    tensor_tensor_scan over the full width resets state between samples
    (and, reversed, scans both samples back-to-front for the bwd cell).
  - T-layout: [token partitions (16 tiles of 128), free = d].
  - a = exp(A[d,n]*dt) via ScalarE activation with per-partition scale;
    u = (dt*x)*B via VectorE TT with B broadcast through a DRAM
    round-trip DMA (step-0 partition access pattern); recurrence via
    VectorE tensor_tensor_scan; sum over n via identity matmuls
    accumulated in PSUM; GEMMs / transposes / LayerNorm statistics
    (ones-vector contractions) on the TensorEngine.
"""

import numpy as np
import ml_dtypes

D = 512
NJ = 4
N = 16
L = 1024
W2 = 2048
WG = 2050
CH_G = [(0, 512), (512, 1024), (1026, 1538), (1538, 2050)]
CH_P = [(0, 512), (512, 1024), (1024, 1536), (1536, 2048)]
CH_C = [(0, 512), (512, 1024), (0, 512), (512, 1024)]
TI_COLG = [(ti // 8) * 1026 + (ti % 8) * 128 for ti in range(16)]
TI_COLP = [(ti // 8) * 1024 + (ti % 8) * 128 for ti in range(16)]

BF16NP = ml_dtypes.bfloat16
_CACHE = {}


def _build_nc():
    import concourse.bacc as bacc
    import concourse.mybir as mybir
    import concourse.tile as tile
    from concourse.bass import AP

    F32 = mybir.dt.float32
    BF = mybir.dt.bfloat16
    Alu = mybir.AluOpType
    Act = mybir.ActivationFunctionType

    nc = bacc.Bacc("TRN2", target_bir_lowering=False, debug=False,
                   num_devices=8)

    def din(name, shape, dt=F32):
        return nc.dram_tensor(name, shape, dt, kind="ExternalInput").ap()

    tok_d = din("tok", [W2, D])
    posT_d = din("posT", [6, L], BF)
    idb_d = din("idb", [128, 128], BF)
    idf_d = din("idf", [128, 128])
    onesb_d = din("onesb", [128, 1], BF)
    onesf_d = din("onesf", [128, 1])
    Wp1_d = din("Wp1", [6, D], BF)
    bp1_d = din("bp1", [D, 1])
    Wp2_d = din("Wp2", [D, D], BF)
    bcond_d = din("bcond", [D, 1])
    ing_d = din("ing", [D, 1])
    cells = ["f", "w"]
    cwd = {}
    for s in cells:
        cwd[s] = dict(
            Wdt=din(f"Wdt{s}", [D, D], BF),
            Wcdt=din(f"Wcdt{s}", [D, D], BF),
            bdt=din(f"bdt{s}", [D, 1]),
            WBC=din(f"WBC{s}", [D, 32], BF),
            bBC=din(f"bBC{s}", [32, 1]),
            Acol=din(f"Acol{s}", [D, N]),
            Dcol=din(f"Dcol{s}", [D, 1]),
        )
    Wmix_d = din("Wmix", [3 * D, D], BF)
    bmix_d = din("bmix", [D, 1])
    outg_d = din("outg", [D, 1])
    outb_d = din("outb", [D, 1])
    ffg_d = din("ffg", [D, 1])
    ffb_d = din("ffb", [D, 1])
    Wf1_d = din("Wf1", [D, 2 * D], BF)
    bf1_d = din("bf1", [2 * D, 1])
    Wf2_d = din("Wf2", [2 * D, D], BF)
    bf2_d = din("bf2", [D, 1])

    out_d = nc.dram_tensor("out", [W2, D], F32, kind="ExternalOutput").ap()

    with tile.TileContext(nc) as tc:
        with (
            tc.tile_pool(name="per", bufs=1) as per,
            tc.tile_pool(name="tokp", bufs=3) as tokp,
            tc.tile_pool(name="scr", bufs=2) as scr,
            tc.tile_pool(name="lnp", bufs=1) as lnp,
            tc.tile_pool(name="bcp", bufs=3) as bcp,
            tc.tile_pool(name="sa", bufs=3) as sap,
            tc.tile_pool(name="su", bufs=3) as sup,
            tc.tile_pool(name="sh", bufs=2) as shp,
            tc.tile_pool(name="hidp", bufs=1) as hidp,
            tc.tile_pool(name="strm", bufs=2) as strm,
            tc.tile_pool(name="psg", bufs=4, space="PSUM") as psg,
            tc.tile_pool(name="psy", bufs=1, space="PSUM") as psy,
            tc.tile_pool(name="dram", bufs=1, space="DRAM") as drp,
        ):
            dma = nc.sync.dma_start
            dmaw = nc.gpsimd.dma_start
            dmag = nc.gpsimd.dma_start

            def loadw(dram_ap, r0, r1, c0, c1, tag, dt=BF):
                t = per.tile([r1 - r0, c1 - c0], dt, tag=tag)
                dmaw(t[:], dram_ap[r0:r1, c0:c1])
                return t

            def load_cols(dram_ap, tag, ntiles=NJ):
                t = per.tile([128, ntiles], F32, tag=tag)
                for j in range(ntiles):
                    dmaw(t[:, j:j + 1], dram_ap[j * 128:(j + 1) * 128, :])
                return t

            # ---- small constants ----
            idb_t = per.tile([128, 128], BF, tag="idb")
            dma(idb_t[:], idb_d[:])
            idf_t = per.tile([128, 128], F32, tag="idf")
            dma(idf_t[:], idf_d[:])
            onesb_t = per.tile([128, 1], BF, tag="onesb")
            dma(onesb_t[:], onesb_d[:])
            onesf_t = per.tile([128, 1], F32, tag="onesf")
            dma(onesf_t[:], onesf_d[:])
            eps_t = per.tile([128, 1], F32, tag="eps")
            nc.vector.memset(eps_t[:], 1e-5)
            ing_t = load_cols(ing_d, "ing")
            bp1_t = load_cols(bp1_d, "bp1")
            bcond_t = load_cols(bcond_d, "bcond")
            outg_t = load_cols(outg_d, "outg")
            outb_t = load_cols(outb_d, "outb")
            ffg_t = load_cols(ffg_d, "ffg")
            ffb_t = load_cols(ffb_d, "ffb")
            bmix_t = load_cols(bmix_d, "bmix")
            bf1_t = load_cols(bf1_d, "bf1", 8)
            bf2_t = load_cols(bf2_d, "bf2")

            # ---- cond MLP (F-layout) ----
            posT_b = per.tile([6, L], BF, tag="posb")
            dma(posT_b[:], posT_d[:])
            Wp1_t = per.tile([6, D], BF, tag="wp1")
            dma(Wp1_t[:], Wp1_d[:])
            c1 = []
            for mj in range(NJ):
                c1t = per.tile([128, L], BF, tag=f"wk{mj}")
                for ch in range(2):
                    ps = psg.tile([128, 512], F32, tag="g")
                    nc.tensor.matmul(
                        ps[:], Wp1_t[:, mj * 128:(mj + 1) * 128],
                        posT_b[:, ch * 512:(ch + 1) * 512],
                        start=True, stop=True)
                    nc.scalar.activation(
                        c1t[:, ch * 512:(ch + 1) * 512], ps[:], Act.Gelu,
                        bias=bp1_t[:, mj:mj + 1])
                c1.append(c1t)
            Wp2_t = [loadw(Wp2_d, kj * 128, (kj + 1) * 128, 0, D,
                           f"wk{4 + kj}") for kj in range(NJ)]
            condA = []
            for mj in range(NJ):
                ct = per.tile([128, L], BF, tag=f"condA{mj}")
                for ch in range(2):
                    ps = psg.tile([128, 512], F32, tag="g")
                    for kj in range(NJ):
                        nc.tensor.matmul(
                            ps[:],
                            Wp2_t[kj][:, mj * 128:(mj + 1) * 128],
                            c1[kj][:, ch * 512:(ch + 1) * 512],
                            start=(kj == 0), stop=(kj == 3))
                    nc.scalar.activation(
                        ct[:, ch * 512:(ch + 1) * 512], ps[:], Act.Identity,
                        bias=bcond_t[:, mj:mj + 1])
                condA.append(ct)

            # ---- input LN stats (T-layout) ----
            s1 = per.tile([128, 16], F32, tag="s1")
            s2 = per.tile([128, 16], F32, tag="s2")
            for ti in range(16):
                tk = tokp.tile([128, D], F32, tag="tok")
                (dma if ti % 2 else dmag)(tk[:],
                                          tok_d[ti * 128:(ti + 1) * 128, :])
                sq = scr.tile([128, D], BF, tag="z2")
                nc.scalar.activation(sq[:], tk[:], Act.Square,
                                     accum_out=s2[:, ti:ti + 1])
                nc.scalar.activation(sq[:], tk[:], Act.Copy,
                                     accum_out=s1[:, ti:ti + 1])
            mean = per.tile([128, 16], F32, tag="mean")
            nc.vector.tensor_scalar_mul(mean[:], s1[:], 1.0 / D)
            nc.vector.tensor_scalar_mul(s2[:], s2[:], 1.0 / D)
            m2 = per.tile([128, 16], F32, tag="m2")
            nc.scalar.activation(m2[:], mean[:], Act.Square)
            nc.vector.tensor_tensor(s2[:], s2[:], m2[:], Alu.subtract)
            nc.scalar.activation(s2[:], s2[:], Act.Sqrt, bias=eps_t[:])
            invstd = per.tile([128, 16], F32, tag="invstd")
            nc.vector.reciprocal(invstd[:], s2[:])

            # ---- scan_inT assembly + tokF spill ----
            xinT = []
            for dj in range(NJ):
                xt = per.tile([128, WG], BF, tag=f"xinT{dj}")
                nc.gpsimd.memset(xt[:, 1024:1026], 0.0)
                xinT.append(xt)
            tokF_dram = drp.tile([D, W2], F32, tag="tokF")
            for ti in range(16):
                tk = tokp.tile([128, D], F32, tag="tok")
                dma(tk[:], tok_d[ti * 128:(ti + 1) * 128, :])
                xn = scr.tile([128, D], BF, tag="xn")
                nc.vector.tensor_scalar(
                    xn[:], tk[:], mean[:, ti:ti + 1], invstd[:, ti:ti + 1],
                    Alu.subtract, Alu.mult)
                colg = TI_COLG[ti]
                colp = TI_COLP[ti]
                ccol = (ti % 8) * 128
                for dj in range(NJ):
                    tp = psg.tile([128, 128], BF, tag="g")
                    nc.tensor.transpose(
                        tp[:], xn[:, dj * 128:(dj + 1) * 128],
                        idb_t[:])
                    nc.vector.scalar_tensor_tensor(
                        xinT[dj][:, colg:colg + 128], tp[:],
                        ing_t[:, dj:dj + 1],
                        condA[dj][:, ccol:ccol + 128],
                        Alu.mult, Alu.add)
                    tp2 = psg.tile([128, 512], F32, tag="g")
                    nc.tensor.transpose(
                        tp2[:, 0:128], tk[:, dj * 128:(dj + 1) * 128],
                        idf_t[:])
                    stg = scr.tile([128, 128], F32, tag="stg")
                    nc.scalar.activation(stg[:], tp2[:, 0:128], Act.Copy)
                    dma(tokF_dram[dj * 128:(dj + 1) * 128,
                                  colp:colp + 128], stg[:])

            # ---- cells ----
            ydram = {}
            for s in cells:
                w = cwd[s]
                Wdt_t = [loadw(w["Wdt"], kj * 128, (kj + 1) * 128, 0, D,
                               f"wk{kj}") for kj in range(NJ)]
                Wcdt_t = [loadw(w["Wcdt"], kj * 128, (kj + 1) * 128, 0, D,
                                f"wk{4 + kj}") for kj in range(NJ)]
                WBC_t = [loadw(w["WBC"], kj * 128, (kj + 1) * 128, 0, 32,
                               f"wk{8 + kj}") for kj in range(NJ)]
                bdt_t = load_cols(w["bdt"], "bdt")
                bBC_t = per.tile([32, 1], F32, tag="bBC")
                dma(bBC_t[:], w["bBC"][:])
                Acol_t = [loadw(w["Acol"], dj * 128, (dj + 1) * 128, 0, N,
                                f"acol{dj}", F32) for dj in range(NJ)]
                Dcol_t = load_cols(w["Dcol"], "dcol")

                dt = []
                vt = []
                for dj in range(NJ):
                    dtt = per.tile([128, WG], BF, tag=f"dt{dj}")
                    for ci in range(4):
                        g0, g1 = CH_G[ci]
                        c0, _ = CH_C[ci]
                        ps = psg.tile([128, 512], F32, tag="g")
                        for kj in range(NJ):
                            nc.tensor.matmul(
                                ps[:],
                                Wdt_t[kj][:, dj * 128:(dj + 1) * 128],
                                xinT[kj][:, g0:g1],
                                start=(kj == 0), stop=False)
                        for kj in range(NJ):
                            nc.tensor.matmul(
                                ps[:],
                                Wcdt_t[kj][:, dj * 128:(dj + 1) * 128],
                                condA[kj][:, c0:c0 + 512],
                                start=False, stop=(kj == 3))
                        et = scr.tile([128, 512], BF, tag="z2")
                        nc.scalar.activation(et[:], ps[:], Act.Exp,
                                             bias=bdt_t[:, dj:dj + 1])
                        nc.scalar.activation(dtt[:, g0:g1], et[:], Act.Ln,
                                             bias=1.0)
                    nc.gpsimd.memset(dtt[:, 1024:1026], 1e9)
                    vtt = per.tile([128, WG], BF, tag=f"v{dj}")
                    nc.vector.tensor_tensor(vtt[:], dtt[:], xinT[dj][:],
                                            Alu.mult)
                    dt.append(dtt)
                    vt.append(vtt)

                BC_sb = per.tile([32, WG], BF, tag="bcsb")
                nc.gpsimd.memset(BC_sb[:, 1024:1026], 0.0)
                for ci in range(4):
                    g0, g1 = CH_G[ci]
                    ps = psg.tile([128, 512], F32, tag="g")
                    for kj in range(NJ):
                        nc.tensor.matmul(
                            ps[0:32, :], WBC_t[kj][:], xinT[kj][:, g0:g1],
                            start=(kj == 0), stop=(kj == 3))
                    nc.scalar.activation(BC_sb[:, g0:g1], ps[0:32, :],
                                         Act.Identity, bias=bBC_t[:])
                BC_dram = drp.tile([32, WG], BF, tag=f"bcd{s}")
                dma(BC_dram[:], BC_sb[:])
                bcd_ap = BC_dram[:]

                y_dram = drp.tile([D, W2], BF, tag=f"yd{s}")
                for dj in range(NJ):
                    yps = psy.tile([128, W2], F32, tag="y")
                    for n in range(N):
                        bct = bcp.tile([128, 2, WG], BF, tag="bc")
                        dma(bct[:], AP(bcd_ap.tensor,
                                        bcd_ap.offset + n * WG,
                                        [[0, 128], [16 * WG, 2], [1, WG]]))
                        brep = bct[:, 0, :]
                        crep = bct[:, 1, :]
                        at = sap.tile([128, WG], BF, tag="a")
                        nc.scalar.activation(
                            at[:], dt[dj][:], Act.Exp,
                            scale=Acol_t[dj][:, n:n + 1])
                        ut = sup.tile([128, WG], BF, tag="u")
                        ueng = nc.gpsimd if n >= 14 else nc.vector
                        ueng.tensor_tensor(ut[:], vt[dj][:], brep,
                                           Alu.mult)
                        ht = shp.tile([128, WG], BF, tag="h")
                        if s == "f":
                            nc.vector.tensor_tensor_scan(
                                ht[:], at[:], ut[:], 0.0,
                                Alu.mult, Alu.add)
                        else:
                            nc.vector.tensor_tensor_scan(
                                ht[:, ::-1], at[:, ::-1], ut[:, ::-1], 0.0,
                                Alu.mult, Alu.add)
                        nc.vector.tensor_tensor(ht[:], ht[:], crep,
                                                Alu.mult)
                        for ci in range(4):
                            g0, g1 = CH_G[ci]
                            p0, p1 = CH_P[ci]
                            nc.tensor.matmul(
                                yps[:, p0:p1], idb_t[:], ht[:, g0:g1],
                                start=(n == 0), stop=(n == N - 1))
                    for ci in range(4):
                        g0, g1 = CH_G[ci]
                        p0, p1 = CH_P[ci]
                        ys = strm.tile([128, 512], BF, tag="lno")
                        nc.vector.scalar_tensor_tensor(
                            ys[:], xinT[dj][:, g0:g1],
                            Dcol_t[:, dj:dj + 1], yps[:, p0:p1],
                            Alu.mult, Alu.add)
                        dma(y_dram[dj * 128:(dj + 1) * 128, p0:p1], ys[:])
                ydram[s] = y_dram

            # ---- mix GEMM + z = tok + mixed (LN1 stats inline) ----
            Wmix_t = [loadw(Wmix_d, kj * 128, (kj + 1) * 128, 0, D,
                            f"wk{kj}") for kj in range(12)]
            z_dram = drp.tile([D, W2], F32, tag="zd")
            s1a = per.tile([1, W2], BF, tag="lns1a")
            s2a = per.tile([1, W2], BF, tag="lns2a")
            for ci in range(4):
                p0, p1 = CH_P[ci]
                c0, _ = CH_C[ci]
                yrhs = []
                for kj in range(8):
                    srcd = ydram["f"] if kj < 4 else ydram["w"]
                    kk = kj % 4
                    yr = per.tile([128, 512], BF, tag=f"ys{kj}")
                    dma(yr[:], srcd[kk * 128:(kk + 1) * 128, p0:p1])
                    yrhs.append(yr)
                ps1 = psg.tile([128, 512], F32, tag="g")
                ps2 = psg.tile([128, 512], F32, tag="g")
                for dj in range(NJ):
                    ps = psg.tile([128, 512], F32, tag="g")
                    for kj in range(8):
                        nc.tensor.matmul(
                            ps[:],
                            Wmix_t[kj][:, dj * 128:(dj + 1) * 128],
                            yrhs[kj][:],
                            start=(kj == 0), stop=False)
                    for kj in range(4):
                        nc.tensor.matmul(
                            ps[:],
                            Wmix_t[8 + kj][:, dj * 128:(dj + 1) * 128],
                            condA[kj][:, c0:c0 + 512],
                            start=False, stop=(kj == 3))
                    tf = strm.tile([128, 512], F32, tag="tf")
                    dma(tf[:], tokF_dram[dj * 128:(dj + 1) * 128, p0:p1])
                    zt = strm.tile([128, 512], F32, tag="zt")
                    nc.vector.scalar_tensor_tensor(
                        zt[:], ps[:], bmix_t[:, dj:dj + 1], tf[:],
                        Alu.add, Alu.add)
                    dma(z_dram[dj * 128:(dj + 1) * 128, p0:p1], zt[:])
                    z2 = scr.tile([128, 512], BF, tag="z2")
                    nc.scalar.activation(z2[:], zt[:], Act.Square)
                    nc.tensor.matmul(ps1[0:1, :], onesf_t[:], zt[:],
                                     start=(dj == 0), stop=(dj == 3))
                    nc.tensor.matmul(ps2[0:1, :], onesb_t[:], z2[:],
                                     start=(dj == 0), stop=(dj == 3))
                nc.scalar.activation(s1a[:, p0:p1], ps1[0:1, :], Act.Copy)
                nc.scalar.activation(s2a[:, p0:p1], ps2[0:1, :], Act.Copy)

            def ln_finalize(s1r, s2r):
                tmp1 = per.tile([1, W2], BF, tag="lntmp1")
                nc.vector.tensor_scalar_mul(s1r[:], s1r[:], 1.0 / D)
                nc.vector.tensor_scalar_mul(s2r[:], s2r[:], 1.0 / D)
                nc.vector.tensor_tensor(tmp1[:], s1r[:], s1r[:], Alu.mult)
                nc.vector.tensor_tensor(s2r[:], s2r[:], tmp1[:],
                                        Alu.subtract)

            def ln_apply(src_dram, s1r, s2r, g_t, b_t, out_dram, out_dt,
                         sink=None):
                # sink: (s1next, s2next) -> accumulate next LN's stats on
                # this LN's output chunks
                for ci in range(4):
                    p0, p1 = CH_P[ci]
                    mb = lnp.tile([128, 512], BF, tag="mb")
                    nc.gpsimd.partition_broadcast(mb[:], s1r[:, p0:p1])
                    ib = lnp.tile([128, 512], BF, tag="ib")
                    nc.gpsimd.partition_broadcast(ib[:], s2r[:, p0:p1])
                    nc.scalar.activation(ib[:], ib[:], Act.Sqrt,
                                         bias=eps_t[:])
                    with nc.allow_low_precision(reason="bf16 1/std"):
                        nc.vector.reciprocal(ib[:], ib[:])
                    if sink is not None:
                        ps1 = psg.tile([128, 512], F32, tag="g")
                        ps2 = psg.tile([128, 512], F32, tag="g")
                    for dj in range(NJ):
                        zt = strm.tile([128, 512], F32, tag="tf")
                        dma(zt[:], src_dram[dj * 128:(dj + 1) * 128, p0:p1])
                        tmp = lnp.tile([128, 512], F32, tag="lntmp")
                        nc.vector.tensor_tensor(tmp[:], zt[:], mb[:],
                                                Alu.subtract)
                        nc.vector.tensor_tensor(tmp[:], tmp[:], ib[:],
                                                Alu.mult)
                        ot = strm.tile([128, 512], out_dt, tag="lno")
                        nc.vector.tensor_scalar(
                            ot[:], tmp[:], g_t[:, dj:dj + 1],
                            b_t[:, dj:dj + 1], Alu.mult, Alu.add)
                        dma(out_dram[dj * 128:(dj + 1) * 128, p0:p1], ot[:])
                        if sink is not None:
                            z2 = scr.tile([128, 512], BF, tag="z2")
                            nc.scalar.activation(z2[:], ot[:], Act.Square)
                            nc.tensor.matmul(ps1[0:1, :], onesf_t[:], ot[:],
                                             start=(dj == 0), stop=(dj == 3))
                            nc.tensor.matmul(ps2[0:1, :], onesb_t[:], z2[:],
                                             start=(dj == 0), stop=(dj == 3))
                    if sink is not None:
                        s1n, s2n = sink
                        nc.scalar.activation(s1n[:, p0:p1], ps1[0:1, :],
                                             Act.Copy)
                        nc.scalar.activation(s2n[:, p0:p1], ps2[0:1, :],
                                             Act.Copy)

            tres_dram = drp.tile([D, W2], F32, tag="trd")
            tn_dram = drp.tile([D, W2], BF, tag="tnd")
            s1b = per.tile([1, W2], BF, tag="lns1b")
            s2b = per.tile([1, W2], BF, tag="lns2b")
            ln_finalize(s1a, s2a)
            ln_apply(z_dram, s1a, s2a, outg_t, outb_t, tres_dram, F32,
                     sink=(s1b, s2b))
            ln_finalize(s1b, s2b)
            ln_apply(tres_dram, s1b, s2b, ffg_t, ffb_t, tn_dram, BF)

            # ---- FFN ----
            Wf1_t = [loadw(Wf1_d, kj * 128, (kj + 1) * 128, 0, 2 * D,
                           f"wk{kj}") for kj in range(NJ)]
            Wf2_t = [loadw(Wf2_d, kj * 128, (kj + 1) * 128, 0, D,
                           f"wk{4 + kj}") for kj in range(8)]
            out_dram_f = drp.tile([D, W2], F32, tag="outF")
            for ci in range(4):
                p0, p1 = CH_P[ci]
                tnc = []
                for kj in range(NJ):
                    tt = per.tile([128, 512], BF, tag=f"ys{kj}")
                    dma(tt[:], tn_dram[kj * 128:(kj + 1) * 128, p0:p1])
                    tnc.append(tt)
                hidc = []
                for hj in range(8):
                    ps = psg.tile([128, 512], F32, tag="g")
                    for kj in range(NJ):
                        nc.tensor.matmul(
                            ps[:],
                            Wf1_t[kj][:, hj * 128:(hj + 1) * 128],
                            tnc[kj][:],
                            start=(kj == 0), stop=(kj == 3))
                    hc = hidp.tile([128, 512], BF, tag=f"hid{hj}")
                    nc.scalar.activation(hc[:], ps[:], Act.Gelu,
                                         bias=bf1_t[:, hj:hj + 1])
                    hidc.append(hc)
                for dj in range(NJ):
                    ps = psg.tile([128, 512], F32, tag="g")
                    for hj in range(8):
                        nc.tensor.matmul(
                            ps[:],
                            Wf2_t[hj][:, dj * 128:(dj + 1) * 128],
                            hidc[hj][:],
                            start=(hj == 0), stop=(hj == 7))
                    tr = strm.tile([128, 512], F32, tag="tf")
                    dma(tr[:], tres_dram[dj * 128:(dj + 1) * 128, p0:p1])
                    of = strm.tile([128, 512], F32, tag="zt")
                    nc.vector.scalar_tensor_tensor(
                        of[:], ps[:], bf2_t[:, dj:dj + 1], tr[:],
                        Alu.add, Alu.add)
                    dma(out_dram_f[dj * 128:(dj + 1) * 128, p0:p1], of[:])

            # ---- transpose back to T-layout and store ----
            for ti in range(16):
                colp = TI_COLP[ti]
                ot = tokp.tile([128, D], F32, tag="ot")
                for dj in range(NJ):
                    ofc = strm.tile([128, 128], F32, tag="tf")
                    dma(ofc[:], out_dram_f[dj * 128:(dj + 1) * 128,
                                           colp:colp + 128])
                    tp = psg.tile([128, 512], F32, tag="g")
                    nc.tensor.transpose(tp[:, 0:128], ofc[:], idf_t[:])
                    nc.scalar.activation(ot[:, dj * 128:(dj + 1) * 128],
                                         tp[:, 0:128], Act.Copy)
                dma(out_d[ti * 128:(ti + 1) * 128, :], ot[:])

    nc.compile()
    return nc


def _host_prep(params):
    p = params

    def np32(x):
        return np.asarray(x, dtype=np.float32)

    def bf(x):
        return np.ascontiguousarray(np.asarray(x, np.float32).astype(BF16NP))

    def col(x):
        return np.ascontiguousarray(np32(x).reshape(-1, 1))

    h = w = 32
    y = np.linspace(-1.0, 1.0, h, dtype=np.float32)
    x = np.linspace(-1.0, 1.0, w, dtype=np.float32)
    yy, xx = np.meshgrid(y, x, indexing="ij")
    r = np.sqrt(xx * xx + yy * yy + 1e-6)
    pos = np.stack([yy, xx, yy * xx, yy * yy, xx * xx, r],
                   axis=-1).reshape(L, 6).astype(np.float32)

    in_b = np32(p["in_b"])
    shared = {
        "posT": np.ascontiguousarray(pos.T).astype(BF16NP),
        "idb": np.eye(128, dtype=np.float32).astype(BF16NP),
        "idf": np.eye(128, dtype=np.float32),
        "onesb": np.ones((128, 1), np.float32).astype(BF16NP),
        "onesf": np.ones((128, 1), np.float32),
        "Wp1": bf(p["W_p1"]),
        "bp1": col(p["b_p1"]),
        "Wp2": bf(p["W_p2"]),
        "bcond": col(np32(p["b_p2"]) + in_b),
        "ing": col(p["in_g"]),
        "Wmix": bf(p["W_mix"]),
        "bmix": col(np32(p["b_mix"])
                    - in_b @ np32(p["W_mix"])[2 * D:3 * D, :]),
        "outg": col(p["out_g"]), "outb": col(p["out_b"]),
        "ffg": col(p["ff_g"]), "ffb": col(p["ff_b"]),
        "Wf1": bf(p["W_f1"]), "bf1": col(p["b_f1"]),
        "Wf2": bf(p["W_f2"]), "bf2": col(p["b_f2"]),
    }
    for s, cell in (("f", p["fwd_cell"]), ("w", p["bwd_cell"])):
        WB = np32(cell["W_B"])
        WC = np32(cell["W_C"])
        shared[f"Wdt{s}"] = bf(cell["W_dt"])
        shared[f"Wcdt{s}"] = bf(cell["W_cdt"])
        shared[f"bdt{s}"] = col(np32(cell["b_dt"])
                                - in_b @ np32(cell["W_cdt"]))
        shared[f"WBC{s}"] = bf(np.concatenate([WB, WC], axis=1))
        shared[f"bBC{s}"] = col(np.concatenate(
            [np32(cell["b_B"]) - in_b @ WB,
             np32(cell["b_C"]) - in_b @ WC]))
        shared[f"Acol{s}"] = np.ascontiguousarray(
            -np.exp(np32(cell["A_log"])))
        shared[f"Dcol{s}"] = col(cell["D"])
    return shared


def kernel(tokens, params, height, width):
    from concourse.bass_utils import run_bass_kernel_spmd

    assert int(height) == 32 and int(width) == 32
    tokens = np.asarray(tokens, dtype=np.float32)
    assert tokens.shape == (16, L, D)

    if "nc" not in _CACHE:
        _CACHE["nc"] = _build_nc()
    nc = _CACHE["nc"]

    shared = _host_prep(params)
    in_maps = []
    for c in range(8):
        m = dict(shared)
        m["tok"] = np.ascontiguousarray(
            tokens[2 * c:2 * c + 2].reshape(W2, D))
        in_maps.append(m)

    res = run_bass_kernel_spmd(nc, in_maps, core_ids=list(range(8)))
    out = np.concatenate(
        [res.results[c]["out"].reshape(2, L, D) for c in range(8)], axis=0)
    return out.astype(np.float32)


# revision 20
# speedup vs baseline: 1.0337x; 1.0337x over previous
"""Bidirectional spatial Mamba block on 8 TRN2 NeuronCores.

Sharding: data-parallel over batch (16 samples -> 2 per core), params
replicated. Each core runs the full block on its 2 local samples.

Device layout:
  - F-layout: [d partitions (4 tiles of 128), free = (b, t)]. Scan-phase
    F tensors are 2050 wide: cols [0,1024) sample 0, [1024,1026) zero
    gap, [1026,2050) sample 1. The gap carries a=0, u=0 so one
    tensor_tensor_scan over the full width resets state between samples
    (and, reversed, scans both samples back-to-front for the bwd cell).
  - T-layout: [token partitions (16 tiles of 128), free = d].
  - a = exp(A[d,n]*dt) via ScalarE activation with per-partition scale;
    u = (dt*x)*B via VectorE TT with B broadcast through a DRAM
    round-trip DMA (step-0 partition access pattern); recurrence via
    VectorE tensor_tensor_scan; sum over n via identity matmuls
    accumulated in PSUM; GEMMs / transposes / LayerNorm statistics
    (ones-vector contractions) on the TensorEngine.
"""

import numpy as np
import ml_dtypes

D = 512
NJ = 4
N = 16
L = 1024
W2 = 2048
WG = 2050
CH_G = [(0, 512), (512, 1024), (1026, 1538), (1538, 2050)]
CH_P = [(0, 512), (512, 1024), (1024, 1536), (1536, 2048)]
CH_C = [(0, 512), (512, 1024), (0, 512), (512, 1024)]
TI_COLG = [(ti // 8) * 1026 + (ti % 8) * 128 for ti in range(16)]
TI_COLP = [(ti // 8) * 1024 + (ti % 8) * 128 for ti in range(16)]

BF16NP = ml_dtypes.bfloat16
_CACHE = {}


def _build_nc():
    import concourse.bacc as bacc
    import concourse.mybir as mybir
    import concourse.tile as tile
    from concourse.bass import AP

    F32 = mybir.dt.float32
    BF = mybir.dt.bfloat16
    Alu = mybir.AluOpType
    Act = mybir.ActivationFunctionType

    nc = bacc.Bacc("TRN2", target_bir_lowering=False, debug=False,
                   num_devices=8)

    def din(name, shape, dt=F32):
        return nc.dram_tensor(name, shape, dt, kind="ExternalInput").ap()

    tok_d = din("tok", [W2, D])
    posT_d = din("posT", [6, L], BF)
    idb_d = din("idb", [128, 128], BF)
    idf_d = din("idf", [128, 128])
    onesb_d = din("onesb", [128, 1], BF)
    onesf_d = din("onesf", [128, 1])
    Wp1_d = din("Wp1", [6, D], BF)
    bp1_d = din("bp1", [D, 1])
    Wp2_d = din("Wp2", [D, D], BF)
    bcond_d = din("bcond", [D, 1])
    ing_d = din("ing", [D, 1])
    cells = ["f", "w"]
    cwd = {}
    for s in cells:
        cwd[s] = dict(
            Wdt=din(f"Wdt{s}", [D, D], BF),
            Wcdt=din(f"Wcdt{s}", [D, D], BF),
            bdt=din(f"bdt{s}", [D, 1]),
            WBC=din(f"WBC{s}", [D, 32], BF),
            bBC=din(f"bBC{s}", [32, 1]),
            Acol=din(f"Acol{s}", [D, N]),
            Dcol=din(f"Dcol{s}", [D, 1]),
        )
    Wmix_d = din("Wmix", [3 * D, D], BF)
    bmix_d = din("bmix", [D, 1])
    outg_d = din("outg", [D, 1])
    outb_d = din("outb", [D, 1])
    ffg_d = din("ffg", [D, 1])
    ffb_d = din("ffb", [D, 1])
    Wf1_d = din("Wf1", [D, 2 * D], BF)
    bf1_d = din("bf1", [2 * D, 1])
    Wf2_d = din("Wf2", [2 * D, D], BF)
    bf2_d = din("bf2", [D, 1])

    out_d = nc.dram_tensor("out", [W2, D], F32, kind="ExternalOutput").ap()

    with tile.TileContext(nc) as tc:
        with (
            tc.tile_pool(name="per", bufs=1) as per,
            tc.tile_pool(name="tokp", bufs=3) as tokp,
            tc.tile_pool(name="scr", bufs=2) as scr,
            tc.tile_pool(name="lnp", bufs=1) as lnp,
            tc.tile_pool(name="bcp", bufs=3) as bcp,
            tc.tile_pool(name="sa", bufs=3) as sap,
            tc.tile_pool(name="su", bufs=3) as sup,
            tc.tile_pool(name="sh", bufs=2) as shp,
            tc.tile_pool(name="hidp", bufs=1) as hidp,
            tc.tile_pool(name="strm", bufs=2) as strm,
            tc.tile_pool(name="psg", bufs=4, space="PSUM") as psg,
            tc.tile_pool(name="psy", bufs=1, space="PSUM") as psy,
            tc.tile_pool(name="dram", bufs=1, space="DRAM") as drp,
        ):
            dma = nc.sync.dma_start
            dmaw = nc.gpsimd.dma_start
            dmag = nc.gpsimd.dma_start

            def loadw(dram_ap, r0, r1, c0, c1, tag, dt=BF):
                t = per.tile([r1 - r0, c1 - c0], dt, tag=tag)
                dmaw(t[:], dram_ap[r0:r1, c0:c1])
                return t

            def load_cols(dram_ap, tag, ntiles=NJ):
                t = per.tile([128, ntiles], F32, tag=tag)
                for j in range(ntiles):
                    dmaw(t[:, j:j + 1], dram_ap[j * 128:(j + 1) * 128, :])
                return t

            # ---- small constants ----
            idb_t = per.tile([128, 128], BF, tag="idb")
            dma(idb_t[:], idb_d[:])
            idf_t = per.tile([128, 128], F32, tag="idf")
            dma(idf_t[:], idf_d[:])
            onesb_t = per.tile([128, 1], BF, tag="onesb")
            dma(onesb_t[:], onesb_d[:])
            onesf_t = per.tile([128, 1], F32, tag="onesf")
            dma(onesf_t[:], onesf_d[:])
            eps_t = per.tile([128, 1], F32, tag="eps")
            nc.vector.memset(eps_t[:], 1e-5)
            ing_t = load_cols(ing_d, "ing")
            bp1_t = load_cols(bp1_d, "bp1")
            bcond_t = load_cols(bcond_d, "bcond")
            outg_t = load_cols(outg_d, "outg")
            outb_t = load_cols(outb_d, "outb")
            ffg_t = load_cols(ffg_d, "ffg")
            ffb_t = load_cols(ffb_d, "ffb")
            bmix_t = load_cols(bmix_d, "bmix")
            bf1_t = load_cols(bf1_d, "bf1", 8)
            bf2_t = load_cols(bf2_d, "bf2")

            # ---- cond MLP (F-layout) ----
            posT_b = per.tile([6, L], BF, tag="posb")
            dma(posT_b[:], posT_d[:])
            Wp1_t = per.tile([6, D], BF, tag="wp1")
            dma(Wp1_t[:], Wp1_d[:])
            c1 = []
            for mj in range(NJ):
                c1t = per.tile([128, L], BF, tag=f"wk{mj}")
                for ch in range(2):
                    ps = psg.tile([128, 512], F32, tag="g")
                    nc.tensor.matmul(
                        ps[:], Wp1_t[:, mj * 128:(mj + 1) * 128],
                        posT_b[:, ch * 512:(ch + 1) * 512],
                        start=True, stop=True)
                    nc.scalar.activation(
                        c1t[:, ch * 512:(ch + 1) * 512], ps[:], Act.Gelu,
                        bias=bp1_t[:, mj:mj + 1])
                c1.append(c1t)
            Wp2_t = [loadw(Wp2_d, kj * 128, (kj + 1) * 128, 0, D,
                           f"wk{4 + kj}") for kj in range(NJ)]
            condA = []
            for mj in range(NJ):
                ct = per.tile([128, L], BF, tag=f"condA{mj}")
                for ch in range(2):
                    ps = psg.tile([128, 512], F32, tag="g")
                    for kj in range(NJ):
                        nc.tensor.matmul(
                            ps[:],
                            Wp2_t[kj][:, mj * 128:(mj + 1) * 128],
                            c1[kj][:, ch * 512:(ch + 1) * 512],
                            start=(kj == 0), stop=(kj == 3))
                    nc.scalar.activation(
                        ct[:, ch * 512:(ch + 1) * 512], ps[:], Act.Identity,
                        bias=bcond_t[:, mj:mj + 1])
                condA.append(ct)

            # ---- input LN stats (T-layout) ----
            s1 = per.tile([128, 16], F32, tag="s1")
            s2 = per.tile([128, 16], F32, tag="s2")
            for ti in range(16):
                tk = tokp.tile([128, D], F32, tag="tok")
                (dma if ti % 2 else dmag)(tk[:],
                                          tok_d[ti * 128:(ti + 1) * 128, :])
                sq = scr.tile([128, D], BF, tag="z2")
                nc.scalar.activation(sq[:], tk[:], Act.Square,
                                     accum_out=s2[:, ti:ti + 1])
                nc.scalar.activation(sq[:], tk[:], Act.Copy,
                                     accum_out=s1[:, ti:ti + 1])
            mean = per.tile([128, 16], F32, tag="mean")
            nc.vector.tensor_scalar_mul(mean[:], s1[:], 1.0 / D)
            nc.vector.tensor_scalar_mul(s2[:], s2[:], 1.0 / D)
            m2 = per.tile([128, 16], F32, tag="m2")
            nc.scalar.activation(m2[:], mean[:], Act.Square)
            nc.vector.tensor_tensor(s2[:], s2[:], m2[:], Alu.subtract)
            nc.scalar.activation(s2[:], s2[:], Act.Sqrt, bias=eps_t[:])
            invstd = per.tile([128, 16], F32, tag="invstd")
            nc.vector.reciprocal(invstd[:], s2[:])

            # ---- scan_inT assembly + tokF spill ----
            xinT = []
            for dj in range(NJ):
                xt = per.tile([128, WG], BF, tag=f"xinT{dj}")
                nc.gpsimd.memset(xt[:, 1024:1026], 0.0)
                xinT.append(xt)
            tokF_dram = drp.tile([D, W2], F32, tag="tokF")
            for ti in range(16):
                tk = tokp.tile([128, D], F32, tag="tok")
                dma(tk[:], tok_d[ti * 128:(ti + 1) * 128, :])
                xn = scr.tile([128, D], BF, tag="xn")
                nc.vector.tensor_scalar(
                    xn[:], tk[:], mean[:, ti:ti + 1], invstd[:, ti:ti + 1],
                    Alu.subtract, Alu.mult)
                colg = TI_COLG[ti]
                colp = TI_COLP[ti]
                ccol = (ti % 8) * 128
                for dj in range(NJ):
                    tp = psg.tile([128, 128], BF, tag="g")
                    nc.tensor.transpose(
                        tp[:], xn[:, dj * 128:(dj + 1) * 128],
                        idb_t[:])
                    nc.vector.scalar_tensor_tensor(
                        xinT[dj][:, colg:colg + 128], tp[:],
                        ing_t[:, dj:dj + 1],
                        condA[dj][:, ccol:ccol + 128],
                        Alu.mult, Alu.add)
                    tp2 = psg.tile([128, 512], F32, tag="g")
                    nc.tensor.transpose(
                        tp2[:, 0:128], tk[:, dj * 128:(dj + 1) * 128],
                        idf_t[:])
                    stg = scr.tile([128, 128], F32, tag="stg")
                    nc.scalar.activation(stg[:], tp2[:, 0:128], Act.Copy)
                    dma(tokF_dram[dj * 128:(dj + 1) * 128,
                                  colp:colp + 128], stg[:])

            # ---- cells ----
            ydram = {}
            for s in cells:
                w = cwd[s]
                Wdt_t = [loadw(w["Wdt"], kj * 128, (kj + 1) * 128, 0, D,
                               f"wk{kj}") for kj in range(NJ)]
                Wcdt_t = [loadw(w["Wcdt"], kj * 128, (kj + 1) * 128, 0, D,
                                f"wk{4 + kj}") for kj in range(NJ)]
                WBC_t = [loadw(w["WBC"], kj * 128, (kj + 1) * 128, 0, 32,
                               f"wk{8 + kj}") for kj in range(NJ)]
                bdt_t = load_cols(w["bdt"], "bdt")
                bBC_t = per.tile([32, 1], F32, tag="bBC")
                dma(bBC_t[:], w["bBC"][:])
                Acol_t = [loadw(w["Acol"], dj * 128, (dj + 1) * 128, 0, N,
                                f"acol{dj}", F32) for dj in range(NJ)]
                Dcol_t = load_cols(w["Dcol"], "dcol")

                dt = []
                vt = []
                for dj in range(NJ):
                    dtt = per.tile([128, WG], BF, tag=f"dt{dj}")
                    for ci in range(4):
                        g0, g1 = CH_G[ci]
                        c0, _ = CH_C[ci]
                        ps = psg.tile([128, 512], F32, tag="g")
                        for kj in range(NJ):
                            nc.tensor.matmul(
                                ps[:],
                                Wdt_t[kj][:, dj * 128:(dj + 1) * 128],
                                xinT[kj][:, g0:g1],
                                start=(kj == 0), stop=False)
                        for kj in range(NJ):
                            nc.tensor.matmul(
                                ps[:],
                                Wcdt_t[kj][:, dj * 128:(dj + 1) * 128],
                                condA[kj][:, c0:c0 + 512],
                                start=False, stop=(kj == 3))
                        et = scr.tile([128, 512], BF, tag="z2")
                        nc.scalar.activation(et[:], ps[:], Act.Exp,
                                             bias=bdt_t[:, dj:dj + 1])
                        nc.scalar.activation(dtt[:, g0:g1], et[:], Act.Ln,
                                             bias=1.0)
                    nc.gpsimd.memset(dtt[:, 1024:1026], 1e9)
                    vtt = per.tile([128, WG], BF, tag=f"v{dj}")
                    nc.vector.tensor_tensor(vtt[:], dtt[:], xinT[dj][:],
                                            Alu.mult)
                    dt.append(dtt)
                    vt.append(vtt)

                BC_sb = per.tile([32, WG], BF, tag="bcsb")
                nc.gpsimd.memset(BC_sb[:, 1024:1026], 0.0)
                for ci in range(4):
                    g0, g1 = CH_G[ci]
                    ps = psg.tile([128, 512], F32, tag="g")
                    for kj in range(NJ):
                        nc.tensor.matmul(
                            ps[0:32, :], WBC_t[kj][:], xinT[kj][:, g0:g1],
                            start=(kj == 0), stop=(kj == 3))
                    nc.scalar.activation(BC_sb[:, g0:g1], ps[0:32, :],
                                         Act.Identity, bias=bBC_t[:])
                BC_dram = drp.tile([32, WG], BF, tag=f"bcd{s}")
                dma(BC_dram[:], BC_sb[:])
                bcd_ap = BC_dram[:]

                y_dram = drp.tile([D, W2], BF, tag=f"yd{s}")
                for dj in range(NJ):
                    yps = psy.tile([128, W2], F32, tag="y")
                    for n in range(N):
                        bct = bcp.tile([128, 2, WG], BF, tag="bc")
                        dma(bct[:], AP(bcd_ap.tensor,
                                        bcd_ap.offset + n * WG,
                                        [[0, 128], [16 * WG, 2], [1, WG]]))
                        brep = bct[:, 0, :]
                        crep = bct[:, 1, :]
                        at = sap.tile([128, WG], BF, tag="a")
                        nc.scalar.activation(
                            at[:], dt[dj][:], Act.Exp,
                            scale=Acol_t[dj][:, n:n + 1])
                        ut = sup.tile([128, WG], BF, tag="u")
                        nc.vector.tensor_tensor(ut[:], vt[dj][:], brep,
                                                Alu.mult)
                        ht = shp.tile([128, WG], BF, tag="h")
                        if s == "f":
                            nc.vector.tensor_tensor_scan(
                                ht[:], at[:], ut[:], 0.0,
                                Alu.mult, Alu.add)
                        else:
                            nc.vector.tensor_tensor_scan(
                                ht[:, ::-1], at[:, ::-1], ut[:, ::-1], 0.0,
                                Alu.mult, Alu.add)
                        nc.vector.tensor_tensor(ht[:], ht[:], crep,
                                                Alu.mult)
                        for ci in range(4):
                            g0, g1 = CH_G[ci]
                            p0, p1 = CH_P[ci]
                            nc.tensor.matmul(
                                yps[:, p0:p1], idb_t[:], ht[:, g0:g1],
                                start=(n == 0), stop=(n == N - 1))
                    for ci in range(4):
                        g0, g1 = CH_G[ci]
                        p0, p1 = CH_P[ci]
                        ys = strm.tile([128, 512], BF, tag="lno")
                        nc.vector.scalar_tensor_tensor(
                            ys[:], xinT[dj][:, g0:g1],
                            Dcol_t[:, dj:dj + 1], yps[:, p0:p1],
                            Alu.mult, Alu.add)
                        dma(y_dram[dj * 128:(dj + 1) * 128, p0:p1], ys[:])
                ydram[s] = y_dram

            # ---- mix GEMM + z = tok + mixed (LN1 stats inline) ----
            Wmix_t = [loadw(Wmix_d, kj * 128, (kj + 1) * 128, 0, D,
                            f"wk{kj}") for kj in range(12)]
            z_dram = drp.tile([D, W2], F32, tag="zd")
            s1a = per.tile([1, W2], BF, tag="lns1a")
            s2a = per.tile([1, W2], BF, tag="lns2a")
            for ci in range(4):
                p0, p1 = CH_P[ci]
                c0, _ = CH_C[ci]
                yrhs = []
                for kj in range(8):
                    srcd = ydram["f"] if kj < 4 else ydram["w"]
                    kk = kj % 4
                    yr = per.tile([128, 512], BF, tag=f"ys{kj}")
                    dma(yr[:], srcd[kk * 128:(kk + 1) * 128, p0:p1])
                    yrhs.append(yr)
                ps1 = psg.tile([128, 512], F32, tag="g")
                ps2 = psg.tile([128, 512], F32, tag="g")
                for dj in range(NJ):
                    ps = psg.tile([128, 512], F32, tag="g")
                    for kj in range(8):
                        nc.tensor.matmul(
                            ps[:],
                            Wmix_t[kj][:, dj * 128:(dj + 1) * 128],
                            yrhs[kj][:],
                            start=(kj == 0), stop=False)
                    for kj in range(4):
                        nc.tensor.matmul(
                            ps[:],
                            Wmix_t[8 + kj][:, dj * 128:(dj + 1) * 128],
                            condA[kj][:, c0:c0 + 512],
                            start=False, stop=(kj == 3))
                    tf = strm.tile([128, 512], F32, tag="tf")
                    dma(tf[:], tokF_dram[dj * 128:(dj + 1) * 128, p0:p1])
                    zt = strm.tile([128, 512], F32, tag="zt")
                    nc.vector.scalar_tensor_tensor(
                        zt[:], ps[:], bmix_t[:, dj:dj + 1], tf[:],
                        Alu.add, Alu.add)
                    dma(z_dram[dj * 128:(dj + 1) * 128, p0:p1], zt[:])
                    z2 = scr.tile([128, 512], BF, tag="z2")
                    nc.scalar.activation(z2[:], zt[:], Act.Square)
                    nc.tensor.matmul(ps1[0:1, :], onesf_t[:], zt[:],
                                     start=(dj == 0), stop=(dj == 3))
                    nc.tensor.matmul(ps2[0:1, :], onesb_t[:], z2[:],
                                     start=(dj == 0), stop=(dj == 3))
                nc.scalar.activation(s1a[:, p0:p1], ps1[0:1, :], Act.Copy)
                nc.scalar.activation(s2a[:, p0:p1], ps2[0:1, :], Act.Copy)

            def ln_finalize(s1r, s2r):
                tmp1 = per.tile([1, W2], BF, tag="lntmp1")
                nc.vector.tensor_scalar_mul(s1r[:], s1r[:], 1.0 / D)
                nc.vector.tensor_scalar_mul(s2r[:], s2r[:], 1.0 / D)
                nc.vector.tensor_tensor(tmp1[:], s1r[:], s1r[:], Alu.mult)
                nc.vector.tensor_tensor(s2r[:], s2r[:], tmp1[:],
                                        Alu.subtract)

            def ln_apply(src_dram, s1r, s2r, g_t, b_t, out_dram, out_dt,
                         sink=None):
                # sink: (s1next, s2next) -> accumulate next LN's stats on
                # this LN's output chunks
                for ci in range(4):
                    p0, p1 = CH_P[ci]
                    mb = lnp.tile([128, 512], BF, tag="mb")
                    nc.gpsimd.partition_broadcast(mb[:], s1r[:, p0:p1])
                    ib = lnp.tile([128, 512], BF, tag="ib")
                    nc.gpsimd.partition_broadcast(ib[:], s2r[:, p0:p1])
                    nc.scalar.activation(ib[:], ib[:], Act.Sqrt,
                                         bias=eps_t[:])
                    with nc.allow_low_precision(reason="bf16 1/std"):
                        nc.vector.reciprocal(ib[:], ib[:])
                    if sink is not None:
                        ps1 = psg.tile([128, 512], F32, tag="g")
                        ps2 = psg.tile([128, 512], F32, tag="g")
                    for dj in range(NJ):
                        zt = strm.tile([128, 512], F32, tag="tf")
                        dma(zt[:], src_dram[dj * 128:(dj + 1) * 128, p0:p1])
                        tmp = lnp.tile([128, 512], F32, tag="lntmp")
                        nc.vector.tensor_tensor(tmp[:], zt[:], mb[:],
                                                Alu.subtract)
                        nc.vector.tensor_tensor(tmp[:], tmp[:], ib[:],
                                                Alu.mult)
                        ot = strm.tile([128, 512], out_dt, tag="lno")
                        nc.vector.tensor_scalar(
                            ot[:], tmp[:], g_t[:, dj:dj + 1],
                            b_t[:, dj:dj + 1], Alu.mult, Alu.add)
                        dma(out_dram[dj * 128:(dj + 1) * 128, p0:p1], ot[:])
                        if sink is not None:
                            z2 = scr.tile([128, 512], BF, tag="z2")
                            nc.scalar.activation(z2[:], ot[:], Act.Square)
                            nc.tensor.matmul(ps1[0:1, :], onesf_t[:], ot[:],
                                             start=(dj == 0), stop=(dj == 3))
                            nc.tensor.matmul(ps2[0:1, :], onesb_t[:], z2[:],
                                             start=(dj == 0), stop=(dj == 3))
                    if sink is not None:
                        s1n, s2n = sink
                        nc.scalar.activation(s1n[:, p0:p1], ps1[0:1, :],
                                             Act.Copy)
                        nc.scalar.activation(s2n[:, p0:p1], ps2[0:1, :],
                                             Act.Copy)

            tres_dram = drp.tile([D, W2], F32, tag="trd")
            tn_dram = drp.tile([D, W2], BF, tag="tnd")
            s1b = per.tile([1, W2], BF, tag="lns1b")
            s2b = per.tile([1, W2], BF, tag="lns2b")
            ln_finalize(s1a, s2a)
            ln_apply(z_dram, s1a, s2a, outg_t, outb_t, tres_dram, F32,
                     sink=(s1b, s2b))
            ln_finalize(s1b, s2b)
            ln_apply(tres_dram, s1b, s2b, ffg_t, ffb_t, tn_dram, BF)

            # ---- FFN ----
            Wf1_t = [loadw(Wf1_d, kj * 128, (kj + 1) * 128, 0, 2 * D,
                           f"wk{kj}") for kj in range(NJ)]
            Wf2_t = [loadw(Wf2_d, kj * 128, (kj + 1) * 128, 0, D,
                           f"wk{4 + kj}") for kj in range(8)]
            out_dram_f = drp.tile([D, W2], F32, tag="outF")
            for ci in range(4):
                p0, p1 = CH_P[ci]
                tnc = []
                for kj in range(NJ):
                    tt = per.tile([128, 512], BF, tag=f"ys{kj}")
                    dma(tt[:], tn_dram[kj * 128:(kj + 1) * 128, p0:p1])
                    tnc.append(tt)
                hidc = []
                for hj in range(8):
                    ps = psg.tile([128, 512], F32, tag="g")
                    for kj in range(NJ):
                        nc.tensor.matmul(
                            ps[:],
                            Wf1_t[kj][:, hj * 128:(hj + 1) * 128],
                            tnc[kj][:],
                            start=(kj == 0), stop=(kj == 3))
                    hc = hidp.tile([128, 512], BF, tag=f"hid{hj}")
                    nc.scalar.activation(hc[:], ps[:], Act.Gelu,
                                         bias=bf1_t[:, hj:hj + 1])
                    hidc.append(hc)
                for dj in range(NJ):
                    ps = psg.tile([128, 512], F32, tag="g")
                    for hj in range(8):
                        nc.tensor.matmul(
                            ps[:],
                            Wf2_t[hj][:, dj * 128:(dj + 1) * 128],
                            hidc[hj][:],
                            start=(hj == 0), stop=(hj == 7))
                    tr = strm.tile([128, 512], F32, tag="tf")
                    dma(tr[:], tres_dram[dj * 128:(dj + 1) * 128, p0:p1])
                    of = strm.tile([128, 512], F32, tag="zt")
                    nc.vector.scalar_tensor_tensor(
                        of[:], ps[:], bf2_t[:, dj:dj + 1], tr[:],
                        Alu.add, Alu.add)
                    dma(out_dram_f[dj * 128:(dj + 1) * 128, p0:p1], of[:])

            # ---- transpose back to T-layout and store ----
            for ti in range(16):
                colp = TI_COLP[ti]
                ot = tokp.tile([128, D], F32, tag="ot")
                for dj in range(NJ):
                    ofc = strm.tile([128, 128], F32, tag="tf")
                    dma(ofc[:], out_dram_f[dj * 128:(dj + 1) * 128,
                                           colp:colp + 128])
                    tp = psg.tile([128, 512], F32, tag="g")
                    nc.tensor.transpose(tp[:, 0:128], ofc[:], idf_t[:])
                    nc.scalar.activation(ot[:, dj * 128:(dj + 1) * 128],
                                         tp[:, 0:128], Act.Copy)
                dma(out_d[ti * 128:(ti + 1) * 128, :], ot[:])

    nc.compile()
    return nc


def _host_prep(params):
    p = params

    def np32(x):
        return np.asarray(x, dtype=np.float32)

    def bf(x):
        return np.ascontiguousarray(np.asarray(x, np.float32).astype(BF16NP))

    def col(x):
        return np.ascontiguousarray(np32(x).reshape(-1, 1))

    h = w = 32
    y = np.linspace(-1.0, 1.0, h, dtype=np.float32)
    x = np.linspace(-1.0, 1.0, w, dtype=np.float32)
    yy, xx = np.meshgrid(y, x, indexing="ij")
    r = np.sqrt(xx * xx + yy * yy + 1e-6)
    pos = np.stack([yy, xx, yy * xx, yy * yy, xx * xx, r],
                   axis=-1).reshape(L, 6).astype(np.float32)

    in_b = np32(p["in_b"])
    shared = {
        "posT": np.ascontiguousarray(pos.T).astype(BF16NP),
        "idb": np.eye(128, dtype=np.float32).astype(BF16NP),
        "idf": np.eye(128, dtype=np.float32),
        "onesb": np.ones((128, 1), np.float32).astype(BF16NP),
        "onesf": np.ones((128, 1), np.float32),
        "Wp1": bf(p["W_p1"]),
        "bp1": col(p["b_p1"]),
        "Wp2": bf(p["W_p2"]),
        "bcond": col(np32(p["b_p2"]) + in_b),
        "ing": col(p["in_g"]),
        "Wmix": bf(p["W_mix"]),
        "bmix": col(np32(p["b_mix"])
                    - in_b @ np32(p["W_mix"])[2 * D:3 * D, :]),
        "outg": col(p["out_g"]), "outb": col(p["out_b"]),
        "ffg": col(p["ff_g"]), "ffb": col(p["ff_b"]),
        "Wf1": bf(p["W_f1"]), "bf1": col(p["b_f1"]),
        "Wf2": bf(p["W_f2"]), "bf2": col(p["b_f2"]),
    }
    for s, cell in (("f", p["fwd_cell"]), ("w", p["bwd_cell"])):
        WB = np32(cell["W_B"])
        WC = np32(cell["W_C"])
        shared[f"Wdt{s}"] = bf(cell["W_dt"])
        shared[f"Wcdt{s}"] = bf(cell["W_cdt"])
        shared[f"bdt{s}"] = col(np32(cell["b_dt"])
                                - in_b @ np32(cell["W_cdt"]))
        shared[f"WBC{s}"] = bf(np.concatenate([WB, WC], axis=1))
        shared[f"bBC{s}"] = col(np.concatenate(
            [np32(cell["b_B"]) - in_b @ WB,
             np32(cell["b_C"]) - in_b @ WC]))
        shared[f"Acol{s}"] = np.ascontiguousarray(
            -np.exp(np32(cell["A_log"])))
        shared[f"Dcol{s}"] = col(cell["D"])
    return shared


def kernel(tokens, params, height, width):
    from concourse.bass_utils import run_bass_kernel_spmd

    assert int(height) == 32 and int(width) == 32
    tokens = np.asarray(tokens, dtype=np.float32)
    assert tokens.shape == (16, L, D)

    if "nc" not in _CACHE:
        _CACHE["nc"] = _build_nc()
    nc = _CACHE["nc"]

    shared = _host_prep(params)
    in_maps = []
    for c in range(8):
        m = dict(shared)
        m["tok"] = np.ascontiguousarray(
            tokens[2 * c:2 * c + 2].reshape(W2, D))
        in_maps.append(m)

    res = run_bass_kernel_spmd(nc, in_maps, core_ids=list(range(8)))
    out = np.concatenate(
        [res.results[c]["out"].reshape(2, L, D) for c in range(8)], axis=0)
    return out.astype(np.float32)
